# revision 21
# baseline (speedup 1.0000x reference)
"""Trainium2 Bass kernel for AttentionWithRotaryPositionalEmbedding.

Problem shapes (hardcoded): x [4, 2048, 512], 8 heads, head dim 64.
Sharding: 8 cores = (batch b = core//2) x (query half = core%2).
Each core computes a [1024, 512] slice of the output; k/v are computed
locally from the full x[b] so no collectives are needed.

Device pipeline per core (matmuls in float32r ~ tf32-precision, full rate):
  phase 1: qkv projections from host-pretransposed x^T; RoPE applied as
           q_rot = q*cos + M2 @ (q*sin) where M2 is the constant pairwise
           rotation permutation (a PE matmul); v scaled by exp(mask) on the
           (otherwise idle) scalar engine during PSUM evacuation.
  phase 2: per head, scores computed transposed sT[k,q] (lhsT = kT head
           slice, K=64); exp on ACT with fused *0.125 scale; attn@v with
           lhsT = [v_h | exp(mask)] (M=65) accumulating over the 16
           k-chunks -- psum row 64 = softmax denominators for free.
           attn@v lags exp by 2 chunks so the in-order PE queue never
           stalls on semaphores (stalls also block LDWEIGHTS hoisting).
  phase 3: denominator replicated via a K=1 ones matmul, fast reciprocal,
           normalize; output projection with bias folded in as a K=1 matmul.
"""

import sys

import numpy as np

if "/opt/trn_rl_repo" not in sys.path:
    sys.path.insert(0, "/opt/trn_rl_repo")

B, N, C = 4, 2048, 512
H, DH = 8, 64
NQ = 1024  # queries per core
P = 128
NCHUNK = N // P  # 16 k chunks
VW = DH + 1  # v columns incl. the emask/ones column
MAX_FPS = np.float32(30.0)

_CACHE = {}


def _host_prep(x, mask, times, Wqkv, Wproj, bproj):
    """Build per-core input maps (numpy only)."""
    x = np.asarray(x, np.float32)
    mask = np.asarray(mask, np.float32)
    times = np.asarray(times, np.float32)
    Wqkv = np.asarray(Wqkv, np.float32)
    Wproj = np.asarray(Wproj, np.float32)
    bproj = np.asarray(bproj, np.float32).reshape(1, C)

    wt = np.ascontiguousarray(Wqkv.T)          # [512, 1536] = [WqT|WkT|WvT]
    wpt = np.ascontiguousarray(Wproj.T)        # [512, 512]

    # pairwise rotation permutation: (M2 @ v)[2i] = -v[2i+1]; [2i+1] = +v[2i]
    M2 = np.zeros((P, P), np.float32)
    for i in range(P // 2):
        M2[2 * i, 2 * i + 1] = -1.0
        M2[2 * i + 1, 2 * i] = 1.0
    m2t = np.ascontiguousarray(M2.T)

    # rotary tables (match reference: all f32 math)
    inv_freq = (np.float32(1.0) /
                (np.float32(10000.0) **
                 (np.arange(0, DH, 2, dtype=np.float32) / np.float32(DH))))  # [32]
    pos = np.round(times * MAX_FPS)  # [B, N] f32, round-half-even like jnp

    in_maps = []
    for core in range(8):
        b, qhalf = core // 2, core % 2
        if qhalf == 0:
            perm = np.arange(N)
        else:
            perm = np.r_[NQ:N, 0:NQ]
        xt = np.ascontiguousarray(x[b].T[:, perm])           # [512, 2048]
        freqs = pos[b][perm][None, :] * inv_freq[:, None]     # [32, 2048] f32
        cos32 = np.cos(freqs.astype(np.float32))
        sin32 = np.sin(freqs.astype(np.float32))
        ridx = (np.arange(P) % DH) // 2                       # row -> pair index
        cose = np.ascontiguousarray(cos32[ridx])              # [128, 2048]
        sine = np.ascontiguousarray(sin32[ridx])
        em = np.exp(mask[b][perm]).astype(np.float32)         # [2048]
        emask = np.ascontiguousarray(em.reshape(NCHUNK, P).T) # [128, 16]
        # cblob: m2t 0:128 | emask 128:144 | ones 144:272 | bias row 272:784
        cblob = np.zeros((P, 784), np.float32)
        cblob[:, 0:128] = m2t
        cblob[:, 128:144] = emask
        cblob[:, 144:272] = 1.0
        cblob[0, 272:784] = bproj[0]
        in_maps.append({
            "xt": xt, "wt": wt, "wpt": wpt,
            "cose": cose, "sine": sine, "cblob": cblob,
        })
    return in_maps


def _build_module():
    import concourse.tile as tile
    import concourse.mybir as mybir
    from concourse import bacc

    f32 = mybir.dt.float32
    f32r = mybir.dt.float32r
    nc = bacc.Bacc(None, target_bir_lowering=False, debug=False)

    xt_d = nc.dram_tensor("xt", [C, N], f32r, kind="ExternalInput")
    wt_d = nc.dram_tensor("wt", [C, 3 * C], f32r, kind="ExternalInput")
    wpt_d = nc.dram_tensor("wpt", [C, C], f32r, kind="ExternalInput")
    cose_d = nc.dram_tensor("cose", [P, N], f32, kind="ExternalInput")
    sine_d = nc.dram_tensor("sine", [P, N], f32, kind="ExternalInput")
    cblob_d = nc.dram_tensor("cblob", [P, 784], f32r, kind="ExternalInput")
    y_d = nc.dram_tensor("y", [NQ, C], f32, kind="ExternalOutput")

    EXP = mybir.ActivationFunctionType.Exp
    COPY = mybir.ActivationFunctionType.Copy

    with tile.TileContext(nc) as tc:
        with (
            tc.tile_pool(name="consts", bufs=1) as consts,
            tc.tile_pool(name="qk", bufs=1) as qk,
            tc.tile_pool(name="vpool", bufs=1) as vpool,
        ):
            # ---- constants (one host-packed f32r blob, one DMA) ----
            # m2t 0:128 | emask 128:144 | ones 144:272 | bias row 272:784
            blob = consts.tile([P, 784], f32r, name="blob")
            nc.sync.dma_start(blob[:], cblob_d.ap())
            m2t_s = blob[:, 0:128]
            emask_s = blob[:, 128:144]
            emask_f = blob[:, 128:144].bitcast(f32)
            ones_s = blob[:, 144:272]
            bias_s = blob[0:1, 272:784]
            # wpt: one [64, 8*512] tile, head h at [:, h*512:(h+1)*512]
            wpt_s = consts.tile([DH, H * C], f32r, name="wpt")
            for hh in range(H):
                nc.sync.dma_start(wpt_s[:, hh * C:(hh + 1) * C],
                                  wpt_d.ap()[hh * DH:(hh + 1) * DH, :])

            # ---- persistent activations ----
            qT = [qk.tile([P, NQ], f32r, name=f"qT{i}") for i in range(4)]
            kT = [qk.tile([P, N], f32r, name=f"kT{i}") for i in range(4)]
            v65 = vpool.tile([P, NCHUNK * H * VW], f32r, name="v65")

            # ================= phase 1: qkv projection + RoPE =================
            ps1_cm = tc.tile_pool(name="ps1", bufs=2, space="PSUM")
            phase1_cm = tc.tile_pool(name="phase1", bufs=1)
            xts_cm = tc.tile_pool(name="xts", bufs=2)
            tmps_cm = tc.tile_pool(name="tmps", bufs=2)
            cep_cm = tc.tile_pool(name="cep", bufs=2)
            ps1 = ps1_cm.__enter__()
            phase1 = phase1_cm.__enter__()
            xts = xts_cm.__enter__()
            tmps = tmps_cm.__enter__()
            cep = cep_cm.__enter__()

            wt_s = [phase1.tile([P, 3 * C], f32r, name=f"wt{i}") for i in range(4)]
            for i in range(4):
                nc.sync.dma_start(wt_s[i][:], wt_d.ap()[i * P:(i + 1) * P, :])

            for nb in range(4):  # n blocks of 512 key positions
                nbs = slice(nb * 512, (nb + 1) * 512)
                ce = cep.tile([P, 1024], f32, name="ce")  # [cos | sin]
                nc.sync.dma_start(ce[:, 0:512], cose_d.ap()[:, nbs])
                nc.sync.dma_start(ce[:, 512:1024], sine_d.ap()[:, nbs])
                xt_t = xts.tile([P, 4, 512], f32r, name="xt_t")
                for ci in range(4):
                    nc.sync.dma_start(
                        xt_t[:, ci, :],
                        xt_d.ap()[ci * P:(ci + 1) * P, nb * 512:(nb + 1) * 512])

                # ---- v projection (natural [n, c] layout) ----
                for tt in range(4):
                    chunk = nb * 4 + tt
                    ps_v = ps1.tile([P, C], f32, name="ps_qkv")
                    for ci in range(4):
                        nc.tensor.matmul(
                            ps_v[:],
                            xt_t[:, ci, tt * P:(tt + 1) * P],
                            wt_s[ci][:, 2 * C:3 * C],
                            start=(ci == 0), stop=(ci == 3))
                    base = chunk * H * VW
                    vv = v65[:, base:base + H * VW].rearrange(
                        "p (h w) -> p h w", w=VW)
                    nc.scalar.activation(
                        vv[:, :, 0:DH],
                        ps_v[:].rearrange("p (h w) -> p h w", w=DH),
                        COPY, scale=emask_f[:, chunk:chunk + 1])
                    nc.vector.tensor_copy(
                        vv[:, :, DH:DH + 1],
                        emask_f[:, chunk:chunk + 1, None].to_broadcast((P, H, 1)))

                # ---- q (only nb<2) and k projections, [c, n] layout + RoPE ----
                species = [("k", C)] if nb >= 2 else [("q", 0), ("k", C)]
                for name_sp, woff in species:
                    for ct in range(4):
                        ps_p = ps1.tile([P, 512], f32, name="ps_qkv")
                        for ci in range(4):
                            nc.tensor.matmul(
                                ps_p[:],
                                wt_s[ci][:, woff + ct * P: woff + (ct + 1) * P],
                                xt_t[:, ci, :],
                                start=(ci == 0), stop=(ci == 3))
                        tmp = tmps.tile([P, 2, 512], f32r, name="tmp")
                        t_c = tmp[:, 0, :]
                        t_s = tmp[:, 1, :]
                        nc.vector.tensor_mul(t_c, ps_p[:], ce[:, 0:512])
                        nc.vector.tensor_mul(t_s, ps_p[:], ce[:, 512:1024])
                        ps_m2 = ps1.tile([P, 512], f32, name="ps_m2")
                        nc.tensor.matmul(ps_m2[:], m2t_s, t_s,
                                         start=True, stop=True)
                        dest = qT[ct] if name_sp == "q" else kT[ct]
                        nc.vector.tensor_add(
                            dest[:, nb * 512:(nb + 1) * 512], t_c, ps_m2[:])

            cep_cm.__exit__(None, None, None)
            tmps_cm.__exit__(None, None, None)
            xts_cm.__exit__(None, None, None)
            phase1_cm.__exit__(None, None, None)
            ps1_cm.__exit__(None, None, None)

            # ================= phase 2: attention per head =================
            ps_score_cm = tc.tile_pool(name="ps_score", bufs=3, space="PSUM")
            ps_av_cm = tc.tile_pool(name="ps_av", bufs=1, space="PSUM")
            outp_cm = tc.tile_pool(name="outp", bufs=1)
            expp_cm = tc.tile_pool(name="expp", bufs=4)
            ps_score = ps_score_cm.__enter__()
            ps_av = ps_av_cm.__enter__()
            outp = outp_cm.__enter__()
            expp = expp_cm.__enter__()
            # per-head attn output scratch [65, 1024]; row 64 = denominators
            sc = [outp.tile([VW, NQ], f32r, name=f"sc{h}") for h in range(H)]

            LAG = 2

            def emit_av(h, c, ps_o, exs):
                voff = (c * H + h) * VW
                for qb in range(2):
                    nc.tensor.matmul(
                        ps_o[:, qb * 512:(qb + 1) * 512],
                        v65[:, voff:voff + VW],
                        exs[c][:, qb * 512:(qb + 1) * 512],
                        start=(c == 0), stop=(c == NCHUNK - 1))

            for h in range(H):
                qt, pb = qT[h // 2], (h % 2) * DH
                kt = kT[h // 2]
                ps_o = ps_av.tile([VW, NQ], f32, name="ps_o")
                exs = {}
                for c in range(NCHUNK):
                    ps_s = ps_score.tile([P, NQ], f32, name="ps_s")
                    for qb in range(2):
                        nc.tensor.matmul(
                            ps_s[:, qb * 512:(qb + 1) * 512],
                            kt[pb:pb + DH, c * P:(c + 1) * P],
                            qt[pb:pb + DH, qb * 512:(qb + 1) * 512],
                            start=True, stop=True)
                    ex = expp.tile([P, NQ], f32r, name="ex")
                    nc.scalar.activation(ex[:], ps_s[:], EXP, scale=0.125)
                    exs[c] = ex
                    if c >= LAG:
                        emit_av(h, c - LAG, ps_o, exs)
                for c in range(NCHUNK - LAG, NCHUNK):
                    emit_av(h, c, ps_o, exs)
                nc.vector.tensor_copy(sc[h][:], ps_o[:])

            expp_cm.__exit__(None, None, None)
            ps_av_cm.__exit__(None, None, None)
            ps_score_cm.__exit__(None, None, None)
            ps3_cm = tc.tile_pool(name="ps3", bufs=2, space="PSUM")
            rrp_cm = tc.tile_pool(name="rrp", bufs=2)
            ypool_cm = tc.tile_pool(name="ypool", bufs=2)
            ps3 = ps3_cm.__enter__()
            rrp = rrp_cm.__enter__()
            ypool = ypool_cm.__enter__()

            # ================= phase 3: normalize + output projection =========
            for h in range(H):
                ps_r = ps3.tile([DH, NQ], f32, name="ps_r")
                for qb in range(2):
                    nc.tensor.matmul(
                        ps_r[:, qb * 512:(qb + 1) * 512],
                        ones_s[DH:DH + 1, 0:DH],
                        sc[h][DH:DH + 1, qb * 512:(qb + 1) * 512],
                        start=True, stop=True)
                rr = rrp.tile([DH, NQ], f32, name="rr")
                nc.vector.reciprocal_approx_fast(rr[:], ps_r[:])
                nc.vector.tensor_mul(
                    sc[h][0:DH, :], sc[h][0:DH, :], rr[:])

            for nbk in range(8):  # output row blocks of 128
                ps_y = ps3.tile([P, C], f32, name="ps_y")
                nc.tensor.matmul(ps_y[:], ones_s[0:1, 0:P], bias_s,
                                 start=True, stop=False)
                for h in range(H):
                    nc.tensor.matmul(
                        ps_y[:],
                        sc[h][0:DH, nbk * P:(nbk + 1) * P],
                        wpt_s[:, h * C:(h + 1) * C],
                        start=False, stop=(h == H - 1))
                y_s = ypool.tile([P, C], f32, name="y_s")
                nc.vector.tensor_copy(y_s[:], ps_y[:])
                nc.sync.dma_start(y_d.ap()[nbk * P:(nbk + 1) * P, :], y_s[:])
            ypool_cm.__exit__(None, None, None)
            rrp_cm.__exit__(None, None, None)
            ps3_cm.__exit__(None, None, None)
            outp_cm.__exit__(None, None, None)

    nc.compile()
    return nc


def _get_module():
    if "nc" not in _CACHE:
        _CACHE["nc"] = _build_module()
    return _CACHE["nc"]


def kernel(x, mask, times, Wqkv, Wproj, bproj, num_cls_token=0, _trace=False):
    from concourse.bass_utils import run_bass_kernel_spmd

    assert int(num_cls_token) == 0, "kernel specialized for num_cls_token=0"
    in_maps = _host_prep(x, mask, times, Wqkv, Wproj, bproj)
    nc = _get_module()
    res = run_bass_kernel_spmd(nc, in_maps, list(range(8)), trace=_trace)
    _CACHE["last_result"] = res

    out = np.empty((B, N, C), np.float32)
    for core in range(8):
        b, qhalf = core // 2, core % 2
        out[b, qhalf * NQ:(qhalf + 1) * NQ, :] = res.results[core]["y"]
    return out
